# revision 65
# baseline (speedup 1.0000x reference)
"""Trainium2 Bass kernel for nn_AuxiliaryTaskPair (segment_reduce).

Computation: for each (batch, pair), mean-pool two spans of sequence_output,
concat the two means, apply a tiny linear [2H -> L]. Returns (logits, labels).

Strategy (data-parallel over batch, 4 examples per core, 8 cores):
  - Span mean pooling as matmul: pooled[span, h] = sum_r pool[r, span] * seq[r, h]
    where pool is a 0/1 membership matrix; exact 1/len scaling is applied
    afterwards per span via tensor_scalar on the PSUM->SBUF copy.
  - "gather" mode (default): only the union of span rows is read from HBM via
    indirect row-gather DMA (~14 MB/core instead of 32 MB/core). Row indices
    are host-built; examples are bin-packed across cores to balance gathered
    row counts (the compiled chunk count kcg is uniform SPMD). Padded index
    slots are RPC and skipped via the gather's bounds_check.
  - The 0/1 pool matrix is generated ON DEVICE per chunk (3 DVE compare ops
    against replicated span bounds) so it costs no HBM traffic.
  - Matmuls run in float32r (4x PE throughput vs f32, plenty of precision for
    this reduction): lhsT = pool chunk [128 rows, 128 spans], rhs = gathered
    seq chunk [128, 512]x2, accumulated in PSUM -> pooled [128 spans, 1024].
  - Transpose pooled via TensorE, final linear as small f32 matmuls
    contracting over H; x-spans and y-spans accumulate into the same
    [64 pairs, 2] PSUM; bias added as a rank-1 (ones x b) matmul.
"""

import os

import numpy as np

import concourse.bass as bass
import concourse.bacc as bacc
import concourse.mybir as mybir
import concourse.tile as tile
from concourse.bass_utils import run_bass_kernel_spmd

B, S, H, P, L = 32, 2048, 1024, 16, 2
N_CORES = 8
EPC = B // N_CORES          # examples per core = 4
RPC = EPC * S               # seq rows per core = 8192
SPANS = EPC * 2 * P         # span columns per core = 128 (x: 0..63, y: 64..127)
PAIRS = EPC * P             # pairs per core = 64

F32 = mybir.dt.float32
F32R = mybir.dt.float32r   # same bits as f32; PE runs 4x faster (N>=256)
I32 = mybir.dt.int32

# Stash of the most recent BassKernelResults (for test harness profiling).
LAST_RESULTS = None


def _build_nc(kcg, gather, n_static=0):
    """kcg: total 128-row chunks per core; the first n_static of them read
    rows [rc*128, rc*128+128) of the shard via static DMA (no index dep, so
    they start immediately and fill the head of the DMA wire), the rest are
    indirect gathers."""
    nc = bacc.Bacc()
    seq = nc.declare_dram_parameter("seq", [RPC, H], F32R, isOutput=False)
    invl = nc.declare_dram_parameter("invl", [128, 1], F32, isOutput=False)
    wcat = nc.declare_dram_parameter("wcat", [128, (H // 128) * 2 * L], F32,
                                     isOutput=False)
    bvec = nc.declare_dram_parameter("bvec", [1, L], F32, isOutput=False)
    if gather:
        # gather row indices (padding = RPC, skipped via bounds_check) and
        # per-span [lo, hi) bounds in global row coords, replicated across
        # partitions ([:, :SPANS] = lo, [:, SPANS:] = hi).
        ridx = nc.declare_dram_parameter("ridx", [128, kcg], I32, isOutput=False)
        bnd = nc.declare_dram_parameter("bnd", [128, 2 * SPANS], F32,
                                        isOutput=False)
    else:
        # full-stream fallback: host-built 0/1 pool, pre-swizzled
        pool = nc.declare_dram_parameter("pool", [128, kcg * SPANS], F32R,
                                         isOutput=False)
    out = nc.declare_dram_parameter("out", [PAIRS, L], F32, isOutput=True)

    from concourse.masks import make_identity

    with tile.TileContext(nc) as tc:
        with (
            tc.tile_pool(name="consts", bufs=1) as consts,
            tc.tile_pool(name="seqp", bufs=10) as seqp,
            tc.tile_pool(name="maskp", bufs=3) as maskp,
            tc.tile_pool(name="big", bufs=1) as big,
            tc.tile_pool(name="acc", bufs=1, space="PSUM") as acc,
            tc.tile_pool(name="tps", bufs=4, space="PSUM") as tpsp,
        ):
            # HWDGE (SP ring) issue order == wire order: ridx first (feeds
            # gather descriptor gen), bnd next (read by the in-loop mask-gen,
            # so it must precede it in program order), then the loop's static
            # chunks; w/b/invl (tail-only consumers) issue after the loop.
            # GPSIMD (Q7) is kept free early for gather descriptor generation.
            if gather:
                ridx_sb = consts.tile([128, kcg], I32)
                nc.sync.dma_start(out=ridx_sb[:], in_=ridx[:])
                bnd_sb = consts.tile([128, 2 * SPANS], F32)
                nc.sync.dma_start(out=bnd_sb[:], in_=bnd[:])
                lo_sb = bnd_sb[:, 0:SPANS]
                hi_sb = bnd_sb[:, SPANS:2 * SPANS]
                ridxf_sb = consts.tile([128, kcg], F32)
                nc.vector.tensor_copy(ridxf_sb[:], ridx_sb[:])
            else:
                pool_sb = consts.tile([128, kcg * SPANS], F32R)
                nc.sync.dma_start(out=pool_sb[:], in_=pool[:])
            w_sb = consts.tile([128, (H // 128) * 2 * L], F32)
            b_sb = consts.tile([1, L], F32)
            invl_sb = consts.tile([128, 1], F32)

            pooled_a = acc.tile([128, 512], F32)
            pooled_b = acc.tile([128, 512], F32)
            for rc in range(kcg):
                st = seqp.tile([128, H], F32R)
                if gather:
                    if rc < n_static:
                        nc.sync.dma_start(
                            out=st[:], in_=seq[rc * 128:(rc + 1) * 128, :]
                        )
                    else:
                        nc.gpsimd.indirect_dma_start(
                            out=st[:],
                            out_offset=None,
                            in_=seq[:],
                            in_offset=bass.IndirectOffsetOnAxis(
                                ap=ridx_sb[:, rc:rc + 1], axis=0
                            ),
                            bounds_check=RPC - 1,
                            oob_is_err=False,
                        )
                    rcol = ridxf_sb[:, rc:rc + 1].to_broadcast([128, SPANS])
                    ge = maskp.tile([128, SPANS], F32)
                    nc.vector.tensor_tensor(out=ge[:], in0=rcol, in1=lo_sb,
                                            op=mybir.AluOpType.is_ge)
                    lt = maskp.tile([128, SPANS], F32)
                    nc.vector.tensor_tensor(out=lt[:], in0=rcol, in1=hi_sb,
                                            op=mybir.AluOpType.is_lt)
                    pt_t = maskp.tile([128, SPANS], F32R)
                    nc.vector.tensor_tensor(out=pt_t[:], in0=ge[:], in1=lt[:],
                                            op=mybir.AluOpType.mult)
                    pt = pt_t[:]
                else:
                    nc.sync.dma_start(
                        out=st[:], in_=seq[rc * 128:(rc + 1) * 128, :]
                    )
                    pt = pool_sb[:, rc * SPANS:(rc + 1) * SPANS]
                nc.tensor.matmul(
                    out=pooled_a[:], lhsT=pt, rhs=st[:, 0:512],
                    start=(rc == 0), stop=(rc == kcg - 1),
                )
                nc.tensor.matmul(
                    out=pooled_b[:], lhsT=pt, rhs=st[:, 512:1024],
                    start=(rc == 0), stop=(rc == kcg - 1),
                )

            nc.sync.dma_start(out=w_sb[:], in_=wcat[:])
            nc.sync.dma_start(out=b_sb[:], in_=bvec[:])
            nc.sync.dma_start(out=invl_sb[:], in_=invl[:])
            identity = consts.tile([128, 128], F32)
            make_identity(nc, identity[:])
            ones = consts.tile([1, PAIRS], F32)
            nc.gpsimd.memset(ones[:], 1.0)

            pooled_sb = big.tile([128, H], F32)
            nc.vector.tensor_scalar_mul(pooled_sb[:, 0:512], pooled_a[:],
                                        invl_sb[:, 0:1])
            nc.scalar.activation(pooled_sb[:, 512:1024], pooled_b[:],
                                 mybir.ActivationFunctionType.Copy,
                                 scale=invl_sb[:, 0:1])

            poolT_sb = big.tile([128, H], F32)
            for hc in range(H // 128):
                tp = tpsp.tile([128, 128], F32)
                nc.tensor.transpose(
                    out=tp[:], in_=pooled_sb[:, hc * 128:(hc + 1) * 128],
                    identity=identity[:],
                )
                dst = poolT_sb[:, hc * 128:(hc + 1) * 128]
                if hc % 2 == 0:
                    nc.vector.tensor_copy(dst, tp[:])
                else:
                    nc.scalar.activation(dst, tp[:],
                                         mybir.ActivationFunctionType.Copy)

            logit_ps = acc.tile([PAIRS, L], F32)
            for hc in range(H // 128):
                base = hc * 128
                nc.tensor.matmul(
                    out=logit_ps[:],
                    lhsT=poolT_sb[:, base:base + PAIRS],
                    rhs=w_sb[:, hc * 2 * L: hc * 2 * L + L],
                    start=(hc == 0), stop=False,
                )
                nc.tensor.matmul(
                    out=logit_ps[:],
                    lhsT=poolT_sb[:, base + PAIRS:base + 2 * PAIRS],
                    rhs=w_sb[:, hc * 2 * L + L: (hc + 1) * 2 * L],
                    start=False, stop=False,
                )
            nc.tensor.matmul(
                out=logit_ps[:], lhsT=ones[:], rhs=b_sb[:],
                start=False, stop=True,
            )

            out_sb = big.tile([PAIRS, L], F32)
            nc.vector.tensor_copy(out_sb[:], logit_ps[:])
            nc.sync.dma_start(out=out[:, :], in_=out_sb[:])

    if not nc.is_finalized():
        nc.finalize()
    return nc


def _assign_examples(sizes):
    """Bin-pack B examples into N_CORES groups of EPC, balancing total sizes.
    Returns list of N_CORES lists of example ids (each length EPC)."""
    order = np.argsort(-np.asarray(sizes))
    groups = [[] for _ in range(N_CORES)]
    totals = [0] * N_CORES
    for e in order:
        cands = [c for c in range(N_CORES) if len(groups[c]) < EPC]
        c = min(cands, key=lambda c: totals[c])
        groups[c].append(int(e))
        totals[c] += int(sizes[e])
    return groups, totals


def _gather_plan(pi):
    """Build per-core gather plans: row indices (ridx), replicated span bounds
    (bnd), and exact per-span 1/len (invl). The first n_static chunks read
    shard rows [0, n_static*128) statically; union rows there (all belong to
    the core's slot-0 example) are excluded from the gather lists.
    Returns (assign, per_core, kcg, n_static)."""
    unions = []
    for e in range(B):
        mask = np.zeros(S, bool)
        for p in range(P):
            mask[pi[e, p, 0]:pi[e, p, 1]] = True
            mask[pi[e, p, 2]:pi[e, p, 3]] = True
        unions.append(np.nonzero(mask)[0].astype(np.int64))
    sizes = [len(u) for u in unions]
    assign, totals = _assign_examples(sizes)

    # Pick n_static (number of leading compile-time chunks covering shard rows
    # [0, ns*128)) minimizing the predicted DMA-wire end. Statics hit the wire
    # at ~2.6us (no index dependency); gathers can't start before ~4.6us
    # (ridx load + completion sem + Q7 descriptor gen). Each core's slot-0
    # example is chosen to maximize union coverage of the static window.
    def plan_ns(ns):
        slot0, kg = [], 0
        for c in range(N_CORES):
            covs = [int((unions[e] < ns * 128).sum()) for e in assign[c]]
            i0 = int(np.argmax(covs))
            slot0.append(i0)
            rem = sum(len(unions[e]) for e in assign[c]) - covs[i0]
            kg = max(kg, (rem + 127) // 128)
        return slot0, kg
    best = None
    for ns in range(0, 5):
        slot0, kg = plan_ns(ns)
        static_end = 2.6 + 1.456 * ns if ns else 0.0
        end = max(static_end, 4.63) + 1.456 * kg
        if best is None or end < best[0]:
            best = (end, ns, slot0, kg)
    _, n_static, slot0, kcg_g = best
    for c in range(N_CORES):
        g = assign[c]
        g[0], g[slot0[c]] = g[slot0[c]], g[0]
    kcg = n_static + kcg_g

    per_core = []
    for c in range(N_CORES):
        # padded slots = RPC: skipped by the gather's bounds_check and outside
        # every span's [lo, hi) (hi <= RPC)
        rows = np.full(kcg * 128, RPC, np.int32)
        rows[:n_static * 128] = np.arange(n_static * 128, dtype=np.int32)
        lo = np.zeros(SPANS, np.float32)
        hi = np.zeros(SPANS, np.float32)
        iv = np.zeros((128, 1), np.float32)
        base = n_static * 128
        for el, e in enumerate(assign[c]):
            u = unions[e]
            if el == 0:
                u = u[u >= n_static * 128]
            n = len(u)
            rows[base:base + n] = (u + el * S).astype(np.int32)
            for p in range(P):
                s1, e1, s2, e2 = (int(pi[e, p, 0]), int(pi[e, p, 1]),
                                  int(pi[e, p, 2]), int(pi[e, p, 3]))
                lo[el * P + p] = s1 + el * S
                hi[el * P + p] = e1 + el * S
                iv[el * P + p, 0] = 1.0 / (e1 - s1)
                lo[PAIRS + el * P + p] = s2 + el * S
                hi[PAIRS + el * P + p] = e2 + el * S
                iv[PAIRS + el * P + p, 0] = 1.0 / (e2 - s2)
            base += n
        bnd = np.concatenate([lo, hi])
        per_core.append({
            "ridx": np.ascontiguousarray(rows.reshape(kcg, 128).T),
            "bnd": np.ascontiguousarray(np.broadcast_to(bnd, (128, 2 * SPANS))),
            "invl": iv,
        })
    return assign, per_core, kcg, n_static


def _stream_plan(pi):
    """Full-stream pooling matrices (identity example assignment)."""
    assign = [[c * EPC + el for el in range(EPC)] for c in range(N_CORES)]
    row = np.arange(S, dtype=np.int64)[None, None, :]
    s1 = pi[..., 0].astype(np.int64)[..., None]
    e1 = pi[..., 1].astype(np.int64)[..., None]
    s2 = pi[..., 2].astype(np.int64)[..., None]
    e2 = pi[..., 3].astype(np.int64)[..., None]
    mx = ((row >= s1) & (row < e1)).astype(np.float32)
    my = ((row >= s2) & (row < e2)).astype(np.float32)
    ivx = 1.0 / (e1 - s1)[..., 0]               # [B, P]
    ivy = 1.0 / (e2 - s2)[..., 0]
    mx = np.transpose(mx, (0, 2, 1))            # [B, S, P]
    my = np.transpose(my, (0, 2, 1))
    pool_list, invl_list = [], []
    for c in range(N_CORES):
        m = np.zeros((RPC, SPANS), np.float32)
        iv = np.zeros((128, 1), np.float32)
        for el in range(EPC):
            e = c * EPC + el
            m[el * S:(el + 1) * S, el * P:(el + 1) * P] = mx[e]
            m[el * S:(el + 1) * S, PAIRS + el * P:PAIRS + (el + 1) * P] = my[e]
            iv[el * P:(el + 1) * P, 0] = ivx[e]
            iv[PAIRS + el * P:PAIRS + (el + 1) * P, 0] = ivy[e]
        pool_list.append(m)
        invl_list.append(iv)
    return assign, pool_list, invl_list


def kernel(**inputs):
    global LAST_RESULTS
    seq = np.ascontiguousarray(np.asarray(inputs["sequence_output"], np.float32))
    pi = np.asarray(inputs["pair_indices"])
    W = np.ascontiguousarray(np.asarray(inputs["W"], np.float32))
    b = np.ascontiguousarray(np.asarray(inputs["b"], np.float32))

    mode = os.environ.get("SEGRED_MODE", "gather")
    # wcat[p, c*2L + l] = [W1.T | W2.T][c*128 + p, l]  (pre-swizzled for SBUF)
    wc = np.concatenate([W[:, :H].T, W[:, H:].T], axis=1)        # [H, 2L]
    wcat = np.ascontiguousarray(
        wc.reshape(H // 128, 128, 2 * L).transpose(1, 0, 2).reshape(128, -1)
    )
    bvec = np.ascontiguousarray(b.reshape(1, L))

    if mode == "gather":
        assign, per_core, kcg, n_static = _gather_plan(pi)
    else:
        assign, pool_list, invl_list = _stream_plan(pi)
        per_core = None
        kcg = RPC // 128
        n_static = 0

    in_maps = []
    for c in range(N_CORES):
        im = {
            "seq": np.ascontiguousarray(
                np.concatenate([seq[e] for e in assign[c]], axis=0)
            ),
            "wcat": wcat,
            "bvec": bvec,
        }
        if per_core is not None:
            im.update(per_core[c])
        else:
            pm = pool_list[c]                   # [kcg*128, SPANS] of 0/1
            im["pool"] = np.ascontiguousarray(
                pm.reshape(kcg, 128, SPANS).transpose(1, 0, 2).reshape(128, -1)
            )
            im["invl"] = invl_list[c]
        in_maps.append(im)

    nc = _build_nc(kcg, gather=(mode == "gather"), n_static=n_static)
    res = run_bass_kernel_spmd(nc, in_maps, core_ids=list(range(N_CORES)))
    LAST_RESULTS = res

    logits = np.zeros((B * P, L), np.float32)
    for c in range(N_CORES):
        o = res.results[c]["out"]                # [PAIRS, L]
        for el, e in enumerate(assign[c]):
            logits[e * P:(e + 1) * P] = o[el * P:(el + 1) * P]
    labels = np.ascontiguousarray(pi[..., 4].reshape(-1).astype(np.int32))
    return logits, labels
